# revision 3
# baseline (speedup 1.0000x reference)
"""Trainium2 Bass kernel for nn_DynaResidualBlockC (hyper-network dynamic
residual block).

Strategy (8 NeuronCores, data-parallel over batch):
  * Each core owns 2 of the 16 samples (samples 2c, 2c+1) and the full
    spatial extent for them.
  * The 51 MB hypernet weight Wk is sharded row-wise 8 ways (3200 rows per
    core after padding 24832 -> 25600).  Each core computes its shard of
    ks = lat @ Wk.T + bk for ALL 16 samples, then one AllToAll hands every
    core exactly its own 2 samples' complete kernel/bias vector.
  * Host-side preprocessing (pure marshalling): Wk rows are pre-scaled by
    the reference's 1/sqrt(fh) / 1/sqrt(fout) constants and permuted so
    that each per-sample conv kernel arrives in transposed ([in,out])
    layout, ready to be DMA'd straight into block-diagonal lhsT tiles.
  * Main loop: both samples are packed on the 128 SBUF partitions
    (64 channels each).  Per spatial tile of 1024 columns:
        psum_in  = W_in.T @ x2                      (PE, f32r)
        w1c/w1s  = sin(psum_in + b_in (+pi/2))      (ACT, fused bias)
        psum_mid = W_mid_c.T @ w1c + W_mid_s.T @ w1s
        w2c/w2s  = sin(psum_mid + b_mid (+pi/2))
        psum_out = W_out_c.T @ w2c + W_out_s.T @ w2s + W_short.T @ x2
        y        = psum_out + (b_out + b_short)     (DVE)
"""
import numpy as np

import concourse.bass as bass
import concourse.bacc as bacc
import concourse.mybir as mybir
from concourse import tile
from concourse.bass_utils import run_bass_kernel_spmd

# ---------------------------------------------------------------- constants
B, FIN, FOUT, FH, H2 = 16, 64, 64, 128, 64
LAT = 512
HH = WW = 192
SP = HH * WW                      # 36864 spatial positions
KTOT, KPAD, SHARD = 24832, 25600, 3200
NCORES = 8
S_TILE = 1024                     # spatial columns per main-loop tile
NT = SP // S_TILE                 # 36
PI_2 = float(np.pi / 2)

F32 = mybir.dt.float32
F32R = mybir.dt.float32r


def _build_perm_scale():
    """orig-row index for each new (device-layout) Wk row, plus row scales."""
    orig = np.full(KPAD, -1, np.int64)
    j = np.arange(4096)
    orig[j] = (j % 64) * 64 + (j // 64)                        # k_in.T
    orig[4096 + j] = 4096 + (j % 64) * 128 + (j // 64)         # k_mid.T rows 0-63
    orig[8192 + j] = 4096 + (j % 64) * 128 + 64 + (j // 64)    # k_mid.T rows 64-127
    orig[12288 + j] = 12288 + (j % 64) * 128 + (j // 64)       # k_out.T rows 0-63
    orig[16384 + j] = 12288 + (j % 64) * 128 + 64 + (j // 64)  # k_out.T rows 64-127
    orig[20480 + j] = 20480 + (j % 64) * 64 + (j // 64)        # k_short.T
    orig[24576:24832] = np.arange(24576, 24832)                # biases, unpermuted
    s = np.ones(KTOT, np.float32)
    s[:12288] = 1.0 / np.sqrt(128.0)
    s[12288:24576] = 1.0 / 8.0
    return orig, s


def _build_nc():
    nc = bacc.Bacc(
        "TRN2",
        target_bir_lowering=False,
        debug=False,
        num_devices=NCORES,
    )
    x_d = nc.dram_tensor("x", [128, SP], F32R, kind="ExternalInput")
    latT_d = nc.dram_tensor("latT", [LAT, B], F32R, kind="ExternalInput")
    wkT_d = nc.dram_tensor("wkT", [LAT, SHARD], F32R, kind="ExternalInput")
    bk_d = nc.dram_tensor("bk", [1, SHARD], F32R, kind="ExternalInput")
    ones_d = nc.dram_tensor("onesr", [1, B], F32R, kind="ExternalInput")
    zeros_d = nc.dram_tensor("zeros", [64, 64], F32R, kind="ExternalInput")
    y_d = nc.dram_tensor("y", [128, SP], F32, kind="ExternalOutput")

    with tile.TileContext(nc) as tc:
        with (
            tc.tile_pool(name="wkt", bufs=1) as wkt_pool,
            tc.tile_pool(name="const", bufs=1) as const_pool,
            tc.tile_pool(name="wts", bufs=1) as w_pool,
            tc.tile_pool(name="dram", bufs=1, space="DRAM") as dram_pool,
            tc.tile_pool(name="psA", bufs=2, space=bass.MemorySpace.PSUM) as psA,
            tc.tile_pool(name="psB", bufs=1, space=bass.MemorySpace.PSUM) as psB,
            tc.tile_pool(name="xin", bufs=3) as x_pool,
            tc.tile_pool(name="waves", bufs=2) as wave_pool,
            tc.tile_pool(name="outs", bufs=3) as out_pool,
        ):
            # ================= prologue: hypernet =================
            lat_tiles = []
            for q in range(4):
                lt = wkt_pool.tile([128, B], F32R, name=f"lat{q}", tag=f"lat{q}")
                nc.sync.dma_start(lt[:], latT_d[128 * q:128 * (q + 1), :])
                lat_tiles.append(lt)
            wkt_tiles = []
            for q in range(4):
                wt = wkt_pool.tile([128, SHARD], F32R, name=f"wkt{q}", tag=f"wkt{q}")
                nc.sync.dma_start(wt[:], wkT_d[128 * q:128 * (q + 1), :])
                wkt_tiles.append(wt)
            ones = const_pool.tile([1, B], F32R, name="ones")
            nc.sync.dma_start(ones[:], ones_d[:])
            bkrow = const_pool.tile([1, SHARD], F32R, name="bkrow")
            nc.sync.dma_start(bkrow[:], bk_d[:])
            ks_sb = const_pool.tile([B, SHARD], F32R, name="ks_sb")

            n0 = 0
            while n0 < SHARD:
                nn = min(512, SHARD - n0)
                ps = psA.tile([B, 512], F32, name="hyps", tag="ps_in")
                for q in range(4):
                    nc.tensor.matmul(
                        ps[:, 0:nn],
                        lat_tiles[q][:],
                        wkt_tiles[q][:, n0:n0 + nn],
                        start=(q == 0),
                        stop=False,
                    )
                nc.tensor.matmul(
                    ps[:, 0:nn],
                    ones[:],
                    bkrow[:, n0:n0 + nn],
                    start=False,
                    stop=True,
                )
                nc.vector.tensor_copy(ks_sb[:, n0:n0 + nn], ps[:, 0:nn])
                n0 += nn

            # ================= exchange: AllToAll =================
            cc_in = dram_pool.tile([B, SHARD], F32R, name="cc_in")
            cc_out = dram_pool.tile([B, SHARD], F32R, name="cc_out")
            nc.gpsimd.dma_start(cc_in[:], ks_sb[:])
            nc.gpsimd.collective_compute(
                "AllToAll",
                mybir.AluOpType.bypass,
                replica_groups=[list(range(NCORES))],
                ins=[cc_in.opt()],
                outs=[cc_out.opt()],
            )

            # ============ per-sample weight/bias assembly ============
            W_in = w_pool.tile([128, 128], F32R, name="W_in")
            W_mid_c = w_pool.tile([128, 128], F32R, name="W_mid_c")
            W_mid_s = w_pool.tile([128, 128], F32R, name="W_mid_s")
            W_out_c = w_pool.tile([128, 128], F32R, name="W_out_c")
            W_out_s = w_pool.tile([128, 128], F32R, name="W_out_s")
            W_short = w_pool.tile([128, 128], F32R, name="W_short")
            regions = [
                (W_in, 0), (W_mid_c, 4096), (W_mid_s, 8192),
                (W_out_c, 12288), (W_out_s, 16384), (W_short, 20480),
            ]
            for Wt, _ in regions:
                # off-diagonal blocks must be zero (block-diagonal lhsT);
                # memset cannot produce "rounded" f32r, so DMA zeros in.
                nc.gpsimd.dma_start(Wt[0:64, 64:128], zeros_d[:])
                nc.gpsimd.dma_start(Wt[64:128, 0:64], zeros_d[:])
            for Wt, base in regions:
                for smp in (0, 1):
                    r = 0
                    while r < 64:
                        flat = base + r * 64
                        shard, col = divmod(flat, SHARD)
                        n = min(64 - r, (SHARD - col) // 64)
                        nc.gpsimd.dma_start(
                            Wt[64 * smp + r:64 * smp + r + n,
                               64 * smp:64 * smp + 64],
                            cc_out[2 * shard + smp:2 * shard + smp + 1,
                                   col:col + 64 * n],
                        )
                        r += n

            bias_flat = const_pool.tile([2, 256], F32R, name="bias_flat")
            # biases live at flat [24576, 24832) -> shard 7, cols 2176:2432
            nc.gpsimd.dma_start(bias_flat[:], cc_out[14:16, 2176:2432])
            vin = const_pool.tile([128, 1], F32R, name="vin")
            vmid = const_pool.tile([128, 1], F32R, name="vmid")
            vout = const_pool.tile([128, 1], F32R, name="vout")
            vsh = const_pool.tile([128, 1], F32R, name="vsh")
            cvin = const_pool.tile([128, 1], F32, name="cvin")
            cvmid = const_pool.tile([128, 1], F32, name="cvmid")
            obias = const_pool.tile([128, 1], F32, name="obias")
            for smp in (0, 1):
                for q, dest in enumerate([vin, vmid, vout, vsh]):
                    nc.gpsimd.dma_start(
                        dest[64 * smp:64 * smp + 64, 0:1],
                        bias_flat[smp:smp + 1, 64 * q:64 * q + 64],
                    )
            nc.vector.tensor_scalar_add(cvin[:], vin[:], PI_2)
            nc.vector.tensor_scalar_add(cvmid[:], vmid[:], PI_2)
            nc.vector.tensor_add(obias[:], vout[:], vsh[:])

            # ================= main loop =================
            SIN = mybir.ActivationFunctionType.Sin
            for t in range(NT):
                c0 = t * S_TILE
                xt = x_pool.tile([128, S_TILE], F32R, name="xt", tag="xt")
                nc.sync.dma_start(xt[:], x_d[:, c0:c0 + S_TILE])

                ps_in = psA.tile([128, S_TILE], F32, name="ps_in", tag="ps_in")
                for ch in range(S_TILE // 512):
                    sl = np.s_[:, ch * 512:(ch + 1) * 512]
                    nc.tensor.matmul(
                        ps_in[sl], W_in[:], xt[sl],
                        start=True, stop=True,
                    )
                w1c = wave_pool.tile([128, S_TILE], F32R, name="w1c", tag="w1c")
                w1s = wave_pool.tile([128, S_TILE], F32R, name="w1s", tag="w1s")
                nc.scalar.activation(w1c[:], ps_in[:], SIN, bias=cvin[:, 0:1])
                nc.scalar.activation(w1s[:], ps_in[:], SIN, bias=vin[:, 0:1])

                ps_mid = psB.tile([128, S_TILE], F32, name="ps_mid", tag="ps_mid")
                for ch in range(S_TILE // 512):
                    sl = np.s_[:, ch * 512:(ch + 1) * 512]
                    nc.tensor.matmul(
                        ps_mid[sl], W_mid_c[:], w1c[sl],
                        start=True, stop=False,
                    )
                    nc.tensor.matmul(
                        ps_mid[sl], W_mid_s[:], w1s[sl],
                        start=False, stop=True,
                    )
                w2c = wave_pool.tile([128, S_TILE], F32R, name="w2c", tag="w2c")
                w2s = wave_pool.tile([128, S_TILE], F32R, name="w2s", tag="w2s")
                nc.scalar.activation(w2c[:], ps_mid[:], SIN, bias=cvmid[:, 0:1])
                nc.scalar.activation(w2s[:], ps_mid[:], SIN, bias=vmid[:, 0:1])

                ps_out = psB.tile([128, S_TILE], F32, name="ps_out", tag="ps_out")
                for ch in range(S_TILE // 512):
                    sl = np.s_[:, ch * 512:(ch + 1) * 512]
                    nc.tensor.matmul(
                        ps_out[sl], W_out_c[:], w2c[sl],
                        start=True, stop=False,
                    )
                    nc.tensor.matmul(
                        ps_out[sl], W_out_s[:], w2s[sl],
                        start=False, stop=False,
                    )
                    nc.tensor.matmul(
                        ps_out[sl], W_short[:], xt[sl],
                        start=False, stop=True,
                    )
                ot = out_pool.tile([128, S_TILE], F32, name="ot", tag="ot")
                nc.vector.tensor_scalar_add(ot[:], ps_out[:], obias[:, 0:1])
                nc.sync.dma_start(y_d[:, c0:c0 + S_TILE], ot[:])

    nc.compile()
    return nc


_NC_CACHE = None


def _get_nc():
    global _NC_CACHE
    if _NC_CACHE is None:
        _NC_CACHE = _build_nc()
    return _NC_CACHE


def kernel(x, lat, Wk, bk, **run_kwargs):
    x = np.ascontiguousarray(np.asarray(x, dtype=np.float32))
    lat = np.ascontiguousarray(np.asarray(lat, dtype=np.float32))
    Wk = np.ascontiguousarray(np.asarray(Wk, dtype=np.float32))
    bk = np.ascontiguousarray(np.asarray(bk, dtype=np.float32))

    orig, s = _build_perm_scale()
    Wk_s = Wk * s[:, None]
    bk_s = bk * s
    Wk_new = np.zeros((KPAD, LAT), np.float32)
    bk_new = np.zeros(KPAD, np.float32)
    valid = orig >= 0
    Wk_new[valid] = Wk_s[orig[valid]]
    bk_new[valid] = bk_s[orig[valid]]
    latT = np.ascontiguousarray(lat.T)

    in_maps = []
    for c in range(NCORES):
        in_maps.append({
            "x": np.ascontiguousarray(
                x[2 * c:2 * c + 2].reshape(128, SP)),
            "latT": latT,
            "wkT": np.ascontiguousarray(
                Wk_new[c * SHARD:(c + 1) * SHARD].T),
            "bk": np.ascontiguousarray(
                bk_new[c * SHARD:(c + 1) * SHARD].reshape(1, SHARD)),
            "onesr": np.ones((1, B), np.float32),
            "zeros": np.zeros((64, 64), np.float32),
        })

    nc = _get_nc()
    res = run_bass_kernel_spmd(nc, in_maps, core_ids=list(range(NCORES)),
                               **run_kwargs)
    y = np.empty((B, FOUT, HH, WW), np.float32)
    for c in range(NCORES):
        y[2 * c:2 * c + 2] = res.results[c]["y"].reshape(2, FOUT, HH, WW)
    if run_kwargs:
        kernel.last_results = res
    return y
